# revision 22
# baseline (speedup 1.0000x reference)
"""Trainium2 Bass kernel for ConvFCNet (3x conv+pool -> int8-fakequant FC + LIF SNN head).

Data-parallel over 8 NeuronCores: batch 512 -> 64 samples/core, weights replicated.

Per-core pipeline (all activations bf16, PSUM accumulation fp32, LIF state fp32):
  conv1 3->32  48x48, pad1 + relu + maxpool2 -> [32, 24, 24]
      im2col (27 = 3c x 9 taps) built by DMA into 4 partition groups of 32,
      4 concurrent diagonal tile_position matmuls (K=27, M=32).
  conv2 32->64 24x24 -> [64, 12, 12]
      im2col over dx only (96 = 32c x 3dx); dy folded into matmul free-dim offsets;
      3 accumulating K=96 matmuls, 2 sample-halves run in parallel via col tiling.
  conv3 64->128 12x12 -> [128, 6, 6]
      im2col over dx: A=[128 = 64c x dx01], B=[64 = c, dx2]; 6 accumulating matmuls.
  FC1 4608->512 per-unit-group (4x128) stationary weights, feat chunks (hw-major) as rhs.
  LIF dynamics (tau=2, vth=1, hard reset) + FC2 512->128 + FC3 128->5, 3 timesteps,
      all in [unit, sample] orientation -> no transposes.
"""

import numpy as np
import ml_dtypes

import concourse.bass as bass
import concourse.bacc as bacc
import concourse.tile as tile
import concourse.mybir as mybir

AF = mybir.ActivationFunctionType
ALU = mybir.AluOpType
BF16 = mybir.dt.bfloat16
F32 = mybir.dt.float32

NCORES = 8
B = 64  # samples per core


def _v(ap, p0, npart, dims, off=0):
    """View into an SBUF/PSUM tile AP: partition slice [p0, p0+npart) + custom free dims."""
    pitch = ap.ap[0][0]
    return bass.AP(
        tensor=ap.tensor,
        offset=ap.offset + p0 * pitch + off,
        ap=[[pitch, npart]] + [list(d) for d in dims],
    )


def _dv(ap, off, dims):
    """View into a DRAM tensor AP with custom dims."""
    return bass.AP(tensor=ap.tensor, offset=ap.offset + off, ap=[list(d) for d in dims])


def _emit(tc, io):
    nc = tc.nc
    from contextlib import ExitStack

    with ExitStack() as ctx:
        # ---------------- persistent buffers + weights ----------------
        wp = ctx.enter_context(tc.tile_pool(name="wts", bufs=1))
        w1sb = wp.tile([108, 128], BF16)
        nc.gpsimd.dma_start(w1sb[:, :], io["w1l"][:, :])
        w2sb = wp.tile([96, 192], BF16)
        nc.gpsimd.dma_start(w2sb[:, :], io["w2l"][:, :])
        w3asb = wp.tile([128, 384], BF16)
        nc.gpsimd.dma_start(w3asb[:, :], io["w3a"][:, :])
        w3bsb = wp.tile([64, 384], BF16)
        nc.gpsimd.dma_start(w3bsb[:, :], io["w3b"][:, :])
        wf2sb = wp.tile([128, 512], BF16)
        nc.gpsimd.dma_start(wf2sb[:, :], io["wf2"][:, :])
        wf3sb = wp.tile([128, 5], BF16)
        nc.gpsimd.dma_start(wf3sb[:, :], io["wf3"][:, :])
        b1sb = wp.tile([128, 1], F32)
        nc.gpsimd.dma_start(b1sb[:, :], io["b1"][:, :])
        b2sb = wp.tile([128, 1], F32)
        nc.gpsimd.dma_start(b2sb[:, :], io["b2"][:, :])
        b3sb = wp.tile([128, 1], F32)
        nc.gpsimd.dma_start(b3sb[:, :], io["b3"][:, :])

        mp = ctx.enter_context(tc.tile_pool(name="main", bufs=1))
        # conv1 pooled output, padded 26x26, partition 32q+c holds samples 16q..16q+15
        xpad2 = mp.tile([128, 16 * 676 + 4], BF16)
        for dims, off in [
            ([[676, 16], [1, 26]], 0),        # top row
            ([[676, 16], [1, 26]], 650),      # bottom row
            ([[676, 16], [26, 26]], 0),       # left col
            ([[676, 16], [26, 26]], 25),      # right col
            ([[1, 4]], 16 * 676),             # tail pad (im2col dx over-read)
        ]:
            nc.gpsimd.memset(_v(xpad2, 0, 128, dims, off), 0.0)
        # conv2 pooled output, padded 14x14, partition 64h+c holds samples 32h..32h+31
        xpad3 = mp.tile([128, 32 * 198 + 4], BF16)
        for dims, off in [
            ([[198, 32], [1, 14]], 0),        # top row
            ([[198, 32], [1, 14]], 182),      # bottom row
            ([[198, 32], [14, 14]], 0),       # left col
            ([[198, 32], [14, 14]], 13),      # right col
            ([[1, 4]], 32 * 198),             # tail pad (im2col dx over-read)
            ([[198, 32], [1, 2]], 196),       # per-sample slack (pitch 198 vs 196)
        ]:
            nc.gpsimd.memset(_v(xpad3, 0, 128, dims, off), 0.0)
        # conv3 pooled output (features): [128c, b*36 + hw]
        feat = mp.tile([128, B * 36], BF16)
        # conv2 im2col buffer: allocated up-front (fresh space, not recycled conv1 SBUF)
        # so its per-quarter DMAs can start as soon as xpad2 sample ranges are written.
        buf96 = mp.tile([96, B * 676 + 4], BF16)
        # LIF state lives in persistent space so its memsets run at t~0 instead
        # of waiting for conv-pool SBUF regions to free up.
        zeros256 = mp.tile([128, 256], F32)
        nc.gpsimd.memset(zeros256[:, :], 0.0)
        v1 = mp.tile([128, 256], F32)
        s1 = mp.tile([128, 256], BF16)
        nc.gpsimd.memset(v1[:, :], 0.0)
        v2 = mp.tile([128, 64], F32)
        nc.gpsimd.memset(v2[:, :], 0.0)
        s2 = mp.tile([128, 64], BF16)
        v3 = mp.tile([5, 64], F32)
        nc.gpsimd.memset(v3[:, :], 0.0)
        acc = mp.tile([5, 64], F32)
        nc.gpsimd.memset(acc[:, :], 0.0)

        # ---------------- conv1 ----------------
        # Block-diagonal stationary [108 = 4g x 27taps, 128 = 4g x 32ch] packs 4
        # sample-groups into one matmul (M=128); moving operand is the
        # host-precomputed im2col c1img [108, (16 slots x 48y x 48x)].
        # FC1 weights tile: chunks are streamed during the conv1/conv2 loop so
        # the 4.7MB load never monopolizes the DMA device right before conv3.
        fcw = ctx.enter_context(tc.tile_pool(name="fcw", bufs=1))
        wf1sb = fcw.tile([128, 18432], BF16)

        # conv1 and conv2 are interleaved per sample-quarter: conv2 block b only
        # needs xpad2 slots 4*(b%4)..4*(b%4)+3, so conv2 (PE-heavy) of quarter
        # qt runs while conv1 (DVE-heavy pooling) of quarter qt+1 proceeds.
        with (
            tc.tile_pool(name="c1imc", bufs=3) as c1i,
            tc.tile_pool(name="c1ps", bufs=4, space="PSUM") as c1p,
            tc.tile_pool(name="c1t", bufs=3) as c1t,
            tc.tile_pool(name="c2ps", bufs=4, space="PSUM") as c2p,
            tc.tile_pool(name="c2t", bufs=3) as c2t,
        ):
            imc_tiles = {}

            def _load_chunk(chunk):
                t = c1i.tile([108, 2 * 2304], BF16, tag="imc")
                for sub in range(2):
                    nc.sync.dma_start(
                        _v(t, 0, 108, [[1, 2304]], sub * 2304),
                        _dv(io["c1img"], (chunk * 2 + sub) * 2304, [[36864, 108], [1, 2304]]),
                    )
                imc_tiles[chunk] = t

            _load_chunk(0)
            _load_chunk(1)

            def _conv1_quarter(qt):
                for half in range(2):
                    chunk = qt * 2 + half
                    imc = imc_tiles.pop(chunk)
                    if chunk + 2 < 8:
                        _load_chunk(chunk + 2)  # prefetch ahead of buf96/wf1 traffic
                    for s in range(2):
                        stg = c1t.tile([128, 576], BF16, tag="stg")
                        slot = chunk * 2 + s
                        for yt in range(6):
                            ps = c1p.tile([128, 384], F32, tag="ps1")
                            rhs = _v(imc, 0, 108, [[1, 384]], s * 2304 + yt * 384)
                            nc.tensor.matmul(
                                ps[:, :], w1sb[0:108, 0:128], rhs, start=True, stop=True
                            )
                            # maxpool 2x2 on (8y, 48x) -> (4y, 24x) into staging
                            nc.vector.tensor_reduce(
                                _v(stg, 0, 128, [[24, 4], [1, 24]], yt * 96),
                                _v(ps, 0, 128, [[96, 4], [2, 24], [48, 2], [1, 2]]),
                                mybir.AxisListType.XY,
                                ALU.max,
                            )
                        dst = _v(xpad2, 0, 128, [[26, 24], [1, 24]], slot * 676 + 27)
                        nc.scalar.activation(dst, _v(stg, 0, 128, [[24, 24], [1, 24]]), AF.Relu, bias=b1sb[:, 0:1])
                    # buf96 for this slot-pair: pipelines with conv1 instead of
                    # batching at quarter end (keeps conv2's PE head unblocked)
                    for j in range(4):
                        bsrc = _v(xpad2, 32 * j, 32, [[1, 3], [1, 2 * 676]], chunk * 2 * 676)
                        bdst = _v(buf96, 0, 96, [[1, 2 * 676]], (j * 16 + chunk * 2) * 676)
                        nc.scalar.dma_start(bdst, bsrc)

            def _conv2_quarter(qt):
                for blk in (qt, qt + 4):
                    stg = c2t.tile([128, 576], BF16, tag="stg")
                    for bi in range(4):
                        b = blk * 4 + bi
                        for yh in range(2):
                            ps = c2p.tile([128, 288], F32, tag="ps2")
                            for h in range(2):
                                for dy in range(3):
                                    rhs = _v(
                                        buf96, 0, 96, [[26, 12], [1, 24]],
                                        (h * 32 + b) * 676 + yh * 312 + dy * 26,
                                    )
                                    nc.tensor.matmul(
                                        ps[64 * h : 64 * h + 64, :],
                                        w2sb[0:96, dy * 64 : dy * 64 + 64],
                                        rhs,
                                        start=(dy == 0),
                                        stop=(dy == 2),
                                        tile_position=(0, 64 * h),
                                    )
                            # pool (12y, 24x) -> (6y, 12x) into staging
                            nc.vector.tensor_reduce(
                                _v(stg, 0, 128, [[12, 6], [1, 12]], bi * 144 + yh * 72),
                                _v(ps, 0, 128, [[48, 6], [2, 12], [24, 2], [1, 2]]),
                                mybir.AxisListType.XY,
                                ALU.max,
                            )
                    dst = _v(xpad3, 0, 128, [[198, 4], [14, 12], [1, 12]], blk * 4 * 198 + 15)
                    nc.scalar.activation(dst, _v(stg, 0, 128, [[144, 4], [12, 12], [1, 12]]), AF.Relu, bias=b2sb[:, 0:1])
                # stream a quarter of the FC1 weights per qt iteration
                nc.gpsimd.dma_start(
                    _v(wf1sb, 0, 128, [[1, 4608]], qt * 4608),
                    _dv(io["wf1"], qt * 4608, [[18432, 128], [1, 4608]]),
                )

            # software-pipeline: conv1 runs 2 quarters ahead at the start so
            # the PE FIFO never stalls on the q0 pool->ACT->buf96 chain.
            _conv1_quarter(0)
            _conv1_quarter(1)
            _conv1_quarter(2)
            _conv2_quarter(0)
            _conv1_quarter(3)
            _conv2_quarter(1)
            _conv2_quarter(2)
            _conv2_quarter(3)

        # ---------------- conv3 (+ FC1 halves interleaved) ----------------
        cur1p = ctx.enter_context(tc.tile_pool(name="cur1p", bufs=1, space="PSUM"))
        cur1 = cur1p.tile([128, 256], F32)
        with (
            tc.tile_pool(name="c3buf", bufs=1) as c3b,
            tc.tile_pool(name="c3ps", bufs=4, space="PSUM") as c3p,
            tc.tile_pool(name="c3t", bufs=3) as c3t,
        ):
            for h in range(2):
                bufA = c3b.tile([128, 32 * 198 + 4], BF16, tag="A")
                bufB = c3b.tile([64, 32 * 198 + 4], BF16, tag="B")
                for ck in range(4):
                    off = ck * 8 * 198
                    nc.scalar.dma_start(
                        _v(bufA, 0, 128, [[1, 8 * 198]], off),
                        _v(xpad3, 64 * h, 64, [[1, 2], [1, 8 * 198]], off),
                    )
                    nc.scalar.dma_start(
                        _v(bufB, 0, 64, [[1, 8 * 198]], off),
                        _v(xpad3, 64 * h, 64, [[1, 8 * 198]], off + 2),
                    )
                for bq in range(4):
                    stg = c3t.tile([128, 288], BF16, tag="stg")
                    for bj in range(4):
                        bp = bq * 4 + bj
                        ps = c3p.tile([128, 288], F32, tag="ps3")
                        for dy in range(3):
                            dims = [[198, 2], [14, 12], [1, 12]]
                            off = bp * 2 * 198 + dy * 14
                            nc.tensor.matmul(
                                ps[:, :], w3asb[0:128, dy * 128 : dy * 128 + 128],
                                _v(bufA, 0, 128, dims, off),
                                start=(dy == 0), stop=False,
                            )
                            nc.tensor.matmul(
                                ps[:, :], w3bsb[0:64, dy * 128 : dy * 128 + 128],
                                _v(bufB, 0, 64, dims, off),
                                start=False, stop=(dy == 2),
                            )
                        # pool (2b, 12y, 12x) -> (2b, 6y, 6x): one XY-reduce per sample
                        for i in range(2):
                            nc.vector.tensor_reduce(
                                _v(stg, 0, 128, [[6, 6], [1, 6]], bj * 72 + i * 36),
                                _v(ps, 0, 128, [[24, 6], [2, 6], [12, 2], [1, 2]], i * 144),
                                mybir.AxisListType.XY,
                                ALU.max,
                            )
                    dst = _v(feat, 0, 128, [[1, 288]], (h * 32 + bq * 8) * 36)
                    nc.scalar.activation(dst, _v(stg, 0, 128, [[1, 288]]), AF.Relu, bias=b3sb[:, 0:1])
                # FC1 half for this h's 32 samples: unit-stationary, output in
                # [unit, sample] orientation (no transposes); overlaps conv3 h=1.
                for k in range(36):
                    for g in range(4):
                        nc.tensor.matmul(
                            cur1[:, 64 * g + 32 * h : 64 * g + 32 * h + 32],
                            wf1sb[:, k * 512 + 128 * g : k * 512 + 128 * (g + 1)],
                            _v(feat, 0, 128, [[36, 32]], k + h * 32 * 36),
                            start=(k == 0),
                            stop=(k == 35),
                        )

        # ---------------- LIF + FC2/FC3 ----------------
        with (
            tc.tile_pool(name="cur2p", bufs=2, space="PSUM") as cur2p,
            tc.tile_pool(name="liftmp", bufs=2) as dtp,
        ):

            def lif_step(v, cur, s_out):
                # v <- v + (cur - v)*0.5 ; s = (v >= 1) ; v <- 0 where s
                n = v.shape[1]
                d = dtp.tile([v.shape[0], n], F32, tag="d", name="d")
                nc.vector.tensor_tensor(d[:, :], cur[:, :], v[:, :], ALU.subtract)
                nc.vector.scalar_tensor_tensor(v[:, :], d[:, :], 0.5, v[:, :], ALU.mult, ALU.add)
                nc.vector.tensor_scalar(s_out[:, :], v[:, :], 1.0, None, ALU.is_ge)
                mask = s_out[:, :].bitcast(mybir.dt.uint16 if s_out.dtype == BF16 else mybir.dt.uint32)
                nc.vector.copy_predicated(v[:, :], mask, zeros256[0 : v.shape[0], 0 : n])

            for t in range(3):
                lif_step(v1, cur1, s1)
                cur2 = cur2p.tile([128, 64], F32, tag="cur2")
                for g in range(4):
                    nc.tensor.matmul(
                        cur2[:, :], wf2sb[:, g * 128 : g * 128 + 128], s1[:, 64 * g : 64 * g + 64],
                        start=(g == 0), stop=(g == 3),
                    )
                lif_step(v2, cur2, s2)
                cur3 = cur2p.tile([5, 64], F32, tag="cur3")
                nc.tensor.matmul(cur3[0:5, :], wf3sb[0:128, 0:5], s2[:, :], start=True, stop=True)
                s3 = dtp.tile([5, 64], F32, tag="s3")
                d3 = dtp.tile([5, 64], F32, tag="d3")
                nc.vector.tensor_tensor(d3[:, :], cur3[0:5, :], v3[:, :], ALU.subtract)
                nc.vector.scalar_tensor_tensor(v3[:, :], d3[:, :], 0.5, v3[:, :], ALU.mult, ALU.add)
                nc.vector.tensor_scalar(s3[:, :], v3[:, :], 1.0, None, ALU.is_ge)
                nc.vector.copy_predicated(v3[:, :], s3[:, :].bitcast(mybir.dt.uint32), zeros256[0:5, 0:64])
                nc.vector.tensor_tensor(acc[:, :], acc[:, :], s3[:, :], ALU.add)

            # acc/3 for acc in {0,1,2,3}: mult by fp32(1/3) matches true division except acc=3
            # (3*0.33333334 = 1.0000001) -> clamp with min(., 1.0) for exactness.
            nc.vector.tensor_scalar(acc[:, :], acc[:, :], float(np.float32(1.0) / np.float32(3.0)), 1.0, ALU.mult, ALU.min)
            nc.sync.dma_start(_dv(io["out"], 0, [[64, 5], [1, 64]]), acc[:, :])


def _build():
    nc = bacc.Bacc("TRN2", target_bir_lowering=False, debug=False, enable_asserts=True)
    io = {}

    def inp(name, shape, dt):
        io[name] = nc.dram_tensor(name, shape, dt, kind="ExternalInput").ap()

    inp("c1img", [108, 36864], BF16)
    inp("w1l", [108, 128], BF16)
    inp("w2l", [96, 192], BF16)
    inp("w3a", [128, 384], BF16)
    inp("w3b", [64, 384], BF16)
    inp("wf1", [128, 18432], BF16)
    inp("wf2", [128, 512], BF16)
    inp("wf3", [128, 5], BF16)
    inp("b1", [128, 1], F32)
    inp("b2", [128, 1], F32)
    inp("b3", [128, 1], F32)
    io["out"] = nc.dram_tensor("out", [5, 64], F32, kind="ExternalOutput").ap()

    import os
    unroll = int(os.environ.get("KERNEL_UNROLL", "1"))
    with tile.TileContext(nc) as tc:
        for _ in range(unroll):
            _emit(tc, io)
    nc.compile()
    return nc


def _fake_quant(w):
    w = np.asarray(w, np.float32)
    scale = np.float32(np.max(np.abs(w)) / np.float32(127.0))
    wq = np.clip(np.round(w / scale), -127.0, 127.0).astype(np.float32) * scale
    return wq.astype(np.float32)


def _bf16(a):
    return np.asarray(a, np.float32).astype(ml_dtypes.bfloat16)


def _prep_weights(conv1_w, conv1_b, conv2_w, conv2_b, conv3_w, conv3_b, W1, W2, W3):
    c1 = np.asarray(conv1_w, np.float32)  # [32, 3, 3, 3]
    c2 = np.asarray(conv2_w, np.float32)  # [64, 32, 3, 3]
    c3 = np.asarray(conv3_w, np.float32)  # [128, 64, 3, 3]

    # conv1 block-diagonal stationary: [108 = (g, c, dy, dx), 128 = (g, co)]
    w1l = np.zeros((108, 128), np.float32)
    wk = c1.transpose(1, 2, 3, 0).reshape(27, 32)  # [(c,dy,dx), co]
    for g in range(4):
        w1l[27 * g : 27 * g + 27, 32 * g : 32 * g + 32] = wk

    w2l = c2.transpose(1, 3, 2, 0).reshape(96, 192)  # [(c,dx), (dy,m)]
    w3x = c3.transpose(1, 3, 2, 0)  # [c, dx, dy, m]
    w3a = w3x[:, 0:2].reshape(128, 384)
    w3b = w3x[:, 2].reshape(64, 384)

    W1q = _fake_quant(W1)  # [512, 4608]
    W2q = _fake_quant(W2)  # [128, 512]
    W3q = _fake_quant(W3)  # [5, 128]

    # [c, k*512 + 128g + u] = W1q[128g + u, c*36 + k]  (unit-stationary FC1)
    wf1 = W1q.reshape(4, 128, 128, 36).transpose(2, 3, 0, 1).reshape(128, 36 * 512)
    wf2 = W2q.T.reshape(4, 128, 128).transpose(1, 0, 2).reshape(128, 512)
    wf3 = W3q.T.copy()  # [128, 5]

    return {
        "w1l": _bf16(w1l),
        "w2l": _bf16(w2l),
        "w3a": _bf16(w3a),
        "w3b": _bf16(w3b),
        "wf1": _bf16(wf1),
        "wf2": _bf16(wf2),
        "wf3": _bf16(wf3),
        "b1": np.tile(np.asarray(conv1_b, np.float32), 4).reshape(128, 1).copy(),
        "b2": np.tile(np.asarray(conv2_b, np.float32), 2).reshape(128, 1).copy(),
        "b3": np.asarray(conv3_b, np.float32).reshape(128, 1).copy(),
    }


_NC = None
LAST_RESULTS = None


def kernel(x, conv1_w, conv1_b, conv2_w, conv2_b, conv3_w, conv3_b, W1, W2, W3, _trace=False):
    global _NC, LAST_RESULTS
    if _NC is None:
        _NC = _build()

    wmap = _prep_weights(conv1_w, conv1_b, conv2_w, conv2_b, conv3_w, conv3_b, W1, W2, W3)

    x = np.asarray(x, np.float32)
    xp = np.zeros((512, 3, 50, 50), np.float32)
    xp[:, :, 1:49, 1:49] = x
    xpb = _bf16(xp)
    in_maps = []
    for i in range(NCORES):
        # host im2col: [108 = (g, c, dy, dx), 36864 = (slot s, y, x)]
        # row value at col (s,y,x) = xpad[64i + 16g + s, c, y+dy, x+dx]
        arr = xpb[B * i : B * (i + 1)].reshape(4, 16, 3, 50, 50)
        c1img = np.empty((108, 36864), ml_dtypes.bfloat16)
        for g in range(4):
            for c in range(3):
                for dy in range(3):
                    for dx in range(3):
                        r = 27 * g + 9 * c + 3 * dy + dx
                        c1img[r] = arr[g, :, c, dy : dy + 48, dx : dx + 48].reshape(-1)
        in_maps.append({"c1img": c1img, **wmap})

    from concourse.bass_utils import run_bass_kernel_spmd

    res = run_bass_kernel_spmd(_NC, in_maps, core_ids=list(range(NCORES)), trace=_trace)
    LAST_RESULTS = res
    out = np.concatenate([np.asarray(res.results[i]["out"]).T for i in range(NCORES)], axis=0)
    return np.ascontiguousarray(out.astype(np.float32))



# revision 25
# speedup vs baseline: 1.0049x; 1.0049x over previous
"""Trainium2 Bass kernel for ConvFCNet (3x conv+pool -> int8-fakequant FC + LIF SNN head).

Data-parallel over 8 NeuronCores: batch 512 -> 64 samples/core, weights replicated.

Per-core pipeline (all activations bf16, PSUM accumulation fp32, LIF state fp32):
  conv1 3->32  48x48, pad1 + relu + maxpool2 -> [32, 24, 24]
      im2col (27 = 3c x 9 taps) built by DMA into 4 partition groups of 32,
      4 concurrent diagonal tile_position matmuls (K=27, M=32).
  conv2 32->64 24x24 -> [64, 12, 12]
      im2col over dx only (96 = 32c x 3dx); dy folded into matmul free-dim offsets;
      3 accumulating K=96 matmuls, 2 sample-halves run in parallel via col tiling.
  conv3 64->128 12x12 -> [128, 6, 6]
      im2col over dx: A=[128 = 64c x dx01], B=[64 = c, dx2]; 6 accumulating matmuls.
  FC1 4608->512 per-unit-group (4x128) stationary weights, feat chunks (hw-major) as rhs.
  LIF dynamics (tau=2, vth=1, hard reset) + FC2 512->128 + FC3 128->5, 3 timesteps,
      all in [unit, sample] orientation -> no transposes.
"""

import numpy as np
import ml_dtypes

import concourse.bass as bass
import concourse.bacc as bacc
import concourse.tile as tile
import concourse.mybir as mybir

AF = mybir.ActivationFunctionType
ALU = mybir.AluOpType
BF16 = mybir.dt.bfloat16
F32 = mybir.dt.float32

NCORES = 8
B = 64  # samples per core


def _v(ap, p0, npart, dims, off=0):
    """View into an SBUF/PSUM tile AP: partition slice [p0, p0+npart) + custom free dims."""
    pitch = ap.ap[0][0]
    return bass.AP(
        tensor=ap.tensor,
        offset=ap.offset + p0 * pitch + off,
        ap=[[pitch, npart]] + [list(d) for d in dims],
    )


def _dv(ap, off, dims):
    """View into a DRAM tensor AP with custom dims."""
    return bass.AP(tensor=ap.tensor, offset=ap.offset + off, ap=[list(d) for d in dims])


def _emit(tc, io):
    nc = tc.nc
    from contextlib import ExitStack

    with ExitStack() as ctx:
        # ---------------- persistent buffers + weights ----------------
        wp = ctx.enter_context(tc.tile_pool(name="wts", bufs=1))
        w1sb = wp.tile([108, 128], BF16)
        nc.gpsimd.dma_start(w1sb[:, :], io["w1l"][:, :])
        w2sb = wp.tile([96, 192], BF16)
        nc.gpsimd.dma_start(w2sb[:, :], io["w2l"][:, :])
        w3asb = wp.tile([128, 384], BF16)
        nc.gpsimd.dma_start(w3asb[:, :], io["w3a"][:, :])
        w3bsb = wp.tile([64, 384], BF16)
        nc.gpsimd.dma_start(w3bsb[:, :], io["w3b"][:, :])
        wf2sb = wp.tile([128, 512], BF16)
        nc.gpsimd.dma_start(wf2sb[:, :], io["wf2"][:, :])
        wf3sb = wp.tile([128, 5], BF16)
        nc.gpsimd.dma_start(wf3sb[:, :], io["wf3"][:, :])
        b1sb = wp.tile([128, 1], F32)
        nc.gpsimd.dma_start(b1sb[:, :], io["b1"][:, :])
        b2sb = wp.tile([128, 1], F32)
        nc.gpsimd.dma_start(b2sb[:, :], io["b2"][:, :])
        b3sb = wp.tile([128, 1], F32)
        nc.gpsimd.dma_start(b3sb[:, :], io["b3"][:, :])

        mp = ctx.enter_context(tc.tile_pool(name="main", bufs=1))
        # conv1 pooled output, padded 26x26, partition 32q+c holds samples 16q..16q+15
        xpad2 = mp.tile([128, 16 * 676 + 4], BF16)
        for dims, off in [
            ([[676, 16], [1, 26]], 0),        # top row
            ([[676, 16], [1, 26]], 650),      # bottom row
            ([[676, 16], [26, 26]], 0),       # left col
            ([[676, 16], [26, 26]], 25),      # right col
            ([[1, 4]], 16 * 676),             # tail pad (im2col dx over-read)
        ]:
            nc.gpsimd.memset(_v(xpad2, 0, 128, dims, off), 0.0)
        # conv2 pooled output, padded 14x14, partition 64h+c holds samples 32h..32h+31
        xpad3 = mp.tile([128, 32 * 198 + 4], BF16)
        for dims, off in [
            ([[198, 32], [1, 14]], 0),        # top row
            ([[198, 32], [1, 14]], 182),      # bottom row
            ([[198, 32], [14, 14]], 0),       # left col
            ([[198, 32], [14, 14]], 13),      # right col
            ([[1, 4]], 32 * 198),             # tail pad (im2col dx over-read)
            ([[198, 32], [1, 2]], 196),       # per-sample slack (pitch 198 vs 196)
        ]:
            nc.gpsimd.memset(_v(xpad3, 0, 128, dims, off), 0.0)
        # conv3 pooled output (features): [128c, b*36 + hw]
        feat = mp.tile([128, B * 36], BF16)
        # conv2 im2col buffer: allocated up-front (fresh space, not recycled conv1 SBUF)
        # so its per-quarter DMAs can start as soon as xpad2 sample ranges are written.
        buf96 = mp.tile([96, B * 676 + 4], BF16)
        # LIF state lives in persistent space so its memsets run at t~0 instead
        # of waiting for conv-pool SBUF regions to free up.
        zeros256 = mp.tile([128, 256], F32)
        nc.gpsimd.memset(zeros256[:, :], 0.0)
        v1 = mp.tile([128, 256], F32)
        s1 = mp.tile([128, 256], BF16)
        nc.gpsimd.memset(v1[:, :], 0.0)
        v2 = mp.tile([128, 64], F32)
        nc.gpsimd.memset(v2[:, :], 0.0)
        s2 = mp.tile([128, 64], BF16)
        v3 = mp.tile([5, 64], F32)
        nc.gpsimd.memset(v3[:, :], 0.0)
        acc = mp.tile([5, 64], F32)
        nc.gpsimd.memset(acc[:, :], 0.0)

        # ---------------- conv1 ----------------
        # Block-diagonal stationary [108 = 4g x 27taps, 128 = 4g x 32ch] packs 4
        # sample-groups into one matmul (M=128); moving operand is the
        # host-precomputed im2col c1img [108, (16 slots x 48y x 48x)].
        # FC1 weights tile: chunks are streamed during the conv1/conv2 loop so
        # the 4.7MB load never monopolizes the DMA device right before conv3.
        fcw = ctx.enter_context(tc.tile_pool(name="fcw", bufs=1))
        wf1sb = fcw.tile([128, 18432], BF16)

        # conv1 and conv2 are interleaved per sample-quarter: conv2 block b only
        # needs xpad2 slots 4*(b%4)..4*(b%4)+3, so conv2 (PE-heavy) of quarter
        # qt runs while conv1 (DVE-heavy pooling) of quarter qt+1 proceeds.
        with (
            tc.tile_pool(name="c1imc", bufs=3) as c1i,
            tc.tile_pool(name="c1ps", bufs=4, space="PSUM") as c1p,
            tc.tile_pool(name="c1t", bufs=3) as c1t,
            tc.tile_pool(name="c2ps", bufs=4, space="PSUM") as c2p,
            tc.tile_pool(name="c2t", bufs=3) as c2t,
        ):
            imc_tiles = {}

            def _load_chunk(chunk):
                t = c1i.tile([108, 2 * 2304], BF16, tag="imc")
                for sub in range(2):
                    nc.sync.dma_start(
                        _v(t, 0, 108, [[1, 2304]], sub * 2304),
                        _dv(io["c1img"], (chunk * 2 + sub) * 2304, [[36864, 108], [1, 2304]]),
                    )
                imc_tiles[chunk] = t

            _load_chunk(0)
            _load_chunk(1)

            def _conv1_quarter(qt):
                for half in range(2):
                    chunk = qt * 2 + half
                    imc = imc_tiles.pop(chunk)
                    if chunk + 2 < 8:
                        _load_chunk(chunk + 2)  # prefetch ahead of buf96/wf1 traffic
                    for s in range(2):
                        stg = c1t.tile([128, 576], BF16, tag="stg")
                        slot = chunk * 2 + s
                        for yt in range(6):
                            ps = c1p.tile([128, 384], F32, tag="ps1")
                            rhs = _v(imc, 0, 108, [[1, 384]], s * 2304 + yt * 384)
                            nc.tensor.matmul(
                                ps[:, :], w1sb[0:108, 0:128], rhs, start=True, stop=True
                            )
                            # maxpool 2x2 on (8y, 48x) -> (4y, 24x) into staging
                            nc.vector.tensor_reduce(
                                _v(stg, 0, 128, [[24, 4], [1, 24]], yt * 96),
                                _v(ps, 0, 128, [[96, 4], [2, 24], [48, 2], [1, 2]]),
                                mybir.AxisListType.XY,
                                ALU.max,
                            )
                        dst = _v(xpad2, 0, 128, [[26, 24], [1, 24]], slot * 676 + 27)
                        nc.scalar.activation(dst, _v(stg, 0, 128, [[24, 24], [1, 24]]), AF.Relu, bias=b1sb[:, 0:1])
                    # buf96 for this slot-pair: pipelines with conv1 instead of
                    # batching at quarter end (keeps conv2's PE head unblocked)
                    for j in range(4):
                        bsrc = _v(xpad2, 32 * j, 32, [[1, 3], [1, 2 * 676]], chunk * 2 * 676)
                        bdst = _v(buf96, 0, 96, [[1, 2 * 676]], (j * 16 + chunk * 2) * 676)
                        nc.scalar.dma_start(bdst, bsrc)

            def _conv2_quarter(qt):
                for blk in (qt, qt + 4):
                    stg = c2t.tile([128, 576], BF16, tag="stg")
                    for bi in range(4):
                        b = blk * 4 + bi
                        for yh in range(2):
                            ps = c2p.tile([128, 288], F32, tag="ps2")
                            for h in range(2):
                                for dy in range(3):
                                    rhs = _v(
                                        buf96, 0, 96, [[26, 12], [1, 24]],
                                        (h * 32 + b) * 676 + yh * 312 + dy * 26,
                                    )
                                    nc.tensor.matmul(
                                        ps[64 * h : 64 * h + 64, :],
                                        w2sb[0:96, dy * 64 : dy * 64 + 64],
                                        rhs,
                                        start=(dy == 0),
                                        stop=(dy == 2),
                                        tile_position=(0, 64 * h),
                                    )
                            # pool (12y, 24x) -> (6y, 12x) into staging
                            nc.vector.tensor_reduce(
                                _v(stg, 0, 128, [[12, 6], [1, 12]], bi * 144 + yh * 72),
                                _v(ps, 0, 128, [[48, 6], [2, 12], [24, 2], [1, 2]]),
                                mybir.AxisListType.XY,
                                ALU.max,
                            )
                    dst = _v(xpad3, 0, 128, [[198, 4], [14, 12], [1, 12]], blk * 4 * 198 + 15)
                    nc.scalar.activation(dst, _v(stg, 0, 128, [[144, 4], [12, 12], [1, 12]]), AF.Relu, bias=b2sb[:, 0:1])
                # stream a quarter of the FC1 weights per qt iteration
                nc.gpsimd.dma_start(
                    _v(wf1sb, 0, 128, [[1, 4608]], qt * 4608),
                    _dv(io["wf1"], qt * 4608, [[18432, 128], [1, 4608]]),
                )

            # software-pipeline with 1-quarter skew: conv1(qt+1) is emitted
            # before conv2(qt) so the PE FIFO never stalls on buf96.
            _conv1_quarter(0)
            for qt in range(4):
                if qt + 1 < 4:
                    _conv1_quarter(qt + 1)
                _conv2_quarter(qt)

        # ---------------- conv3 (+ FC1 halves interleaved) ----------------
        cur1p = ctx.enter_context(tc.tile_pool(name="cur1p", bufs=1, space="PSUM"))
        cur1 = cur1p.tile([128, 256], F32)
        with (
            tc.tile_pool(name="c3buf", bufs=1) as c3b,
            tc.tile_pool(name="c3ps", bufs=4, space="PSUM") as c3p,
            tc.tile_pool(name="c3t", bufs=3) as c3t,
        ):
            for h in range(2):
                bufA = c3b.tile([128, 32 * 198 + 4], BF16, tag="A")
                bufB = c3b.tile([64, 32 * 198 + 4], BF16, tag="B")
                for ck in range(4):
                    off = ck * 8 * 198
                    nc.sync.dma_start(
                        _v(bufA, 0, 128, [[1, 8 * 198]], off),
                        _v(xpad3, 64 * h, 64, [[1, 2], [1, 8 * 198]], off),
                    )
                    nc.sync.dma_start(
                        _v(bufB, 0, 64, [[1, 8 * 198]], off),
                        _v(xpad3, 64 * h, 64, [[1, 8 * 198]], off + 2),
                    )
                for bq in range(4):
                    stg = c3t.tile([128, 288], BF16, tag="stg")
                    for bj in range(4):
                        bp = bq * 4 + bj
                        ps = c3p.tile([128, 288], F32, tag="ps3")
                        for dy in range(3):
                            dims = [[198, 2], [14, 12], [1, 12]]
                            off = bp * 2 * 198 + dy * 14
                            nc.tensor.matmul(
                                ps[:, :], w3asb[0:128, dy * 128 : dy * 128 + 128],
                                _v(bufA, 0, 128, dims, off),
                                start=(dy == 0), stop=False,
                            )
                            nc.tensor.matmul(
                                ps[:, :], w3bsb[0:64, dy * 128 : dy * 128 + 128],
                                _v(bufB, 0, 64, dims, off),
                                start=False, stop=(dy == 2),
                            )
                        # pool (2b, 12y, 12x) -> (2b, 6y, 6x): one XY-reduce per sample
                        for i in range(2):
                            nc.vector.tensor_reduce(
                                _v(stg, 0, 128, [[6, 6], [1, 6]], bj * 72 + i * 36),
                                _v(ps, 0, 128, [[24, 6], [2, 6], [12, 2], [1, 2]], i * 144),
                                mybir.AxisListType.XY,
                                ALU.max,
                            )
                    dst = _v(feat, 0, 128, [[1, 288]], (h * 32 + bq * 8) * 36)
                    nc.scalar.activation(dst, _v(stg, 0, 128, [[1, 288]]), AF.Relu, bias=b3sb[:, 0:1])
                # FC1 half for this h's 32 samples: unit-stationary, output in
                # [unit, sample] orientation (no transposes); overlaps conv3 h=1.
                for k in range(36):
                    for g in range(4):
                        nc.tensor.matmul(
                            cur1[:, 64 * g + 32 * h : 64 * g + 32 * h + 32],
                            wf1sb[:, k * 512 + 128 * g : k * 512 + 128 * (g + 1)],
                            _v(feat, 0, 128, [[36, 32]], k + h * 32 * 36),
                            start=(k == 0),
                            stop=(k == 35),
                        )

        # ---------------- LIF + FC2/FC3 ----------------
        with (
            tc.tile_pool(name="cur2p", bufs=2, space="PSUM") as cur2p,
            tc.tile_pool(name="liftmp", bufs=2) as dtp,
        ):

            def lif_half(v, cur, s_out, vw):
                # v <- v + (cur - v)*0.5 ; s = (v >= 1) ; v <- 0 where s
                # vw: list of column views (strided half-slices share one AP)
                d = dtp.tile([v.shape[0], vw[0][1]], F32, tag=f"d{vw[0][0]}", name="d")
                dv = _v(d, 0, v.shape[0], [[1, vw[0][1]]])
                cv = bass.AP(tensor=cur.tensor, offset=cur.offset + vw[1][0], ap=[list(cur.ap[0])] + vw[1][2])
                vv = bass.AP(tensor=v.tensor, offset=v.offset + vw[1][0], ap=[list(v.ap[0])] + vw[1][2])
                sv = bass.AP(tensor=s_out.tensor, offset=s_out.offset + vw[1][0], ap=[list(s_out.ap[0])] + vw[1][2])
                nc.vector.tensor_tensor(dv, cv, vv, ALU.subtract)
                nc.vector.scalar_tensor_tensor(vv, dv, 0.5, vv, ALU.mult, ALU.add)
                nc.vector.tensor_scalar(sv, vv, 1.0, None, ALU.is_ge)
                mask = sv.bitcast(mybir.dt.uint16 if s_out.dtype == BF16 else mybir.dt.uint32)
                nc.vector.copy_predicated(vv, mask, zeros256[0 : v.shape[0], 0 : vw[0][1]])

            # two independent sample-half pipelines per timestep: FC matmuls of
            # one half overlap LIF vector ops of the other.
            for t in range(3):
                cur2 = cur2p.tile([128, 64], F32, tag="cur2")
                cur3 = cur2p.tile([5, 64], F32, tag="cur3")
                for hh in range(2):
                    # v1/s1 half: strided cols {64g+32hh .. +32}
                    lif_half(v1, cur1, s1, [(hh, 128), (32 * hh, None, [[64, 4], [1, 32]])])
                    for g in range(4):
                        nc.tensor.matmul(
                            cur2[:, 32 * hh : 32 * hh + 32],
                            wf2sb[:, g * 128 : g * 128 + 128],
                            s1[:, 64 * g + 32 * hh : 64 * g + 32 * hh + 32],
                            start=(g == 0), stop=(g == 3),
                        )
                for hh in range(2):
                    lif_half(v2, cur2, s2, [(hh, 32), (32 * hh, None, [[1, 32]])])
                    nc.tensor.matmul(
                        cur3[0:5, 32 * hh : 32 * hh + 32], wf3sb[0:128, 0:5],
                        s2[:, 32 * hh : 32 * hh + 32], start=True, stop=True,
                    )
                for hh in range(2):
                    s3 = dtp.tile([5, 32], F32, tag=f"s3{hh}")
                    d3 = dtp.tile([5, 32], F32, tag=f"d3{hh}")
                    c3v = _v(cur3, 0, 5, [[1, 32]], 32 * hh)
                    v3v = _v(v3, 0, 5, [[1, 32]], 32 * hh)
                    nc.vector.tensor_tensor(d3[:, :], c3v, v3v, ALU.subtract)
                    nc.vector.scalar_tensor_tensor(v3v, d3[:, :], 0.5, v3v, ALU.mult, ALU.add)
                    nc.vector.tensor_scalar(s3[:, :], v3v, 1.0, None, ALU.is_ge)
                    nc.vector.copy_predicated(v3v, s3[:, :].bitcast(mybir.dt.uint32), zeros256[0:5, 0:32])
                    nc.vector.tensor_tensor(_v(acc, 0, 5, [[1, 32]], 32 * hh), _v(acc, 0, 5, [[1, 32]], 32 * hh), s3[:, :], ALU.add)

            # acc/3 for acc in {0,1,2,3}: mult by fp32(1/3) matches true division except acc=3
            # (3*0.33333334 = 1.0000001) -> clamp with min(., 1.0) for exactness.
            nc.vector.tensor_scalar(acc[:, :], acc[:, :], float(np.float32(1.0) / np.float32(3.0)), 1.0, ALU.mult, ALU.min)
            nc.sync.dma_start(_dv(io["out"], 0, [[64, 5], [1, 64]]), acc[:, :])


def _build():
    nc = bacc.Bacc("TRN2", target_bir_lowering=False, debug=False, enable_asserts=True)
    io = {}

    def inp(name, shape, dt):
        io[name] = nc.dram_tensor(name, shape, dt, kind="ExternalInput").ap()

    inp("c1img", [108, 36864], BF16)
    inp("w1l", [108, 128], BF16)
    inp("w2l", [96, 192], BF16)
    inp("w3a", [128, 384], BF16)
    inp("w3b", [64, 384], BF16)
    inp("wf1", [128, 18432], BF16)
    inp("wf2", [128, 512], BF16)
    inp("wf3", [128, 5], BF16)
    inp("b1", [128, 1], F32)
    inp("b2", [128, 1], F32)
    inp("b3", [128, 1], F32)
    io["out"] = nc.dram_tensor("out", [5, 64], F32, kind="ExternalOutput").ap()

    import os
    unroll = int(os.environ.get("KERNEL_UNROLL", "1"))
    with tile.TileContext(nc) as tc:
        for _ in range(unroll):
            _emit(tc, io)
    nc.compile()
    return nc


def _fake_quant(w):
    w = np.asarray(w, np.float32)
    scale = np.float32(np.max(np.abs(w)) / np.float32(127.0))
    wq = np.clip(np.round(w / scale), -127.0, 127.0).astype(np.float32) * scale
    return wq.astype(np.float32)


def _bf16(a):
    return np.asarray(a, np.float32).astype(ml_dtypes.bfloat16)


def _prep_weights(conv1_w, conv1_b, conv2_w, conv2_b, conv3_w, conv3_b, W1, W2, W3):
    c1 = np.asarray(conv1_w, np.float32)  # [32, 3, 3, 3]
    c2 = np.asarray(conv2_w, np.float32)  # [64, 32, 3, 3]
    c3 = np.asarray(conv3_w, np.float32)  # [128, 64, 3, 3]

    # conv1 block-diagonal stationary: [108 = (g, c, dy, dx), 128 = (g, co)]
    w1l = np.zeros((108, 128), np.float32)
    wk = c1.transpose(1, 2, 3, 0).reshape(27, 32)  # [(c,dy,dx), co]
    for g in range(4):
        w1l[27 * g : 27 * g + 27, 32 * g : 32 * g + 32] = wk

    w2l = c2.transpose(1, 3, 2, 0).reshape(96, 192)  # [(c,dx), (dy,m)]
    w3x = c3.transpose(1, 3, 2, 0)  # [c, dx, dy, m]
    w3a = w3x[:, 0:2].reshape(128, 384)
    w3b = w3x[:, 2].reshape(64, 384)

    W1q = _fake_quant(W1)  # [512, 4608]
    W2q = _fake_quant(W2)  # [128, 512]
    W3q = _fake_quant(W3)  # [5, 128]

    # [c, k*512 + 128g + u] = W1q[128g + u, c*36 + k]  (unit-stationary FC1)
    wf1 = W1q.reshape(4, 128, 128, 36).transpose(2, 3, 0, 1).reshape(128, 36 * 512)
    wf2 = W2q.T.reshape(4, 128, 128).transpose(1, 0, 2).reshape(128, 512)
    wf3 = W3q.T.copy()  # [128, 5]

    return {
        "w1l": _bf16(w1l),
        "w2l": _bf16(w2l),
        "w3a": _bf16(w3a),
        "w3b": _bf16(w3b),
        "wf1": _bf16(wf1),
        "wf2": _bf16(wf2),
        "wf3": _bf16(wf3),
        "b1": np.tile(np.asarray(conv1_b, np.float32), 4).reshape(128, 1).copy(),
        "b2": np.tile(np.asarray(conv2_b, np.float32), 2).reshape(128, 1).copy(),
        "b3": np.asarray(conv3_b, np.float32).reshape(128, 1).copy(),
    }


_NC = None
LAST_RESULTS = None


def kernel(x, conv1_w, conv1_b, conv2_w, conv2_b, conv3_w, conv3_b, W1, W2, W3, _trace=False):
    global _NC, LAST_RESULTS
    if _NC is None:
        _NC = _build()

    wmap = _prep_weights(conv1_w, conv1_b, conv2_w, conv2_b, conv3_w, conv3_b, W1, W2, W3)

    x = np.asarray(x, np.float32)
    xp = np.zeros((512, 3, 50, 50), np.float32)
    xp[:, :, 1:49, 1:49] = x
    xpb = _bf16(xp)
    in_maps = []
    for i in range(NCORES):
        # host im2col: [108 = (g, c, dy, dx), 36864 = (slot s, y, x)]
        # row value at col (s,y,x) = xpad[64i + 16g + s, c, y+dy, x+dx]
        arr = xpb[B * i : B * (i + 1)].reshape(4, 16, 3, 50, 50)
        c1img = np.empty((108, 36864), ml_dtypes.bfloat16)
        for g in range(4):
            for c in range(3):
                for dy in range(3):
                    for dx in range(3):
                        r = 27 * g + 9 * c + 3 * dy + dx
                        c1img[r] = arr[g, :, c, dy : dy + 48, dx : dx + 48].reshape(-1)
        in_maps.append({"c1img": c1img, **wmap})

    from concourse.bass_utils import run_bass_kernel_spmd

    res = run_bass_kernel_spmd(_NC, in_maps, core_ids=list(range(NCORES)), trace=_trace)
    LAST_RESULTS = res
    out = np.concatenate([np.asarray(res.results[i]["out"]).T for i in range(NCORES)], axis=0)
    return np.ascontiguousarray(out.astype(np.float32))



# revision 26
# speedup vs baseline: 1.0268x; 1.0218x over previous
"""Trainium2 Bass kernel for ConvFCNet (3x conv+pool -> int8-fakequant FC + LIF SNN head).

Data-parallel over 8 NeuronCores: batch 512 -> 64 samples/core, weights replicated.

Per-core pipeline (all activations bf16, PSUM accumulation fp32, LIF state fp32):
  conv1 3->32  48x48, pad1 + relu + maxpool2 -> [32, 24, 24]
      im2col (27 = 3c x 9 taps) built by DMA into 4 partition groups of 32,
      4 concurrent diagonal tile_position matmuls (K=27, M=32).
  conv2 32->64 24x24 -> [64, 12, 12]
      im2col over dx only (96 = 32c x 3dx); dy folded into matmul free-dim offsets;
      3 accumulating K=96 matmuls, 2 sample-halves run in parallel via col tiling.
  conv3 64->128 12x12 -> [128, 6, 6]
      im2col over dx: A=[128 = 64c x dx01], B=[64 = c, dx2]; 6 accumulating matmuls.
  FC1 4608->512 per-unit-group (4x128) stationary weights, feat chunks (hw-major) as rhs.
  LIF dynamics (tau=2, vth=1, hard reset) + FC2 512->128 + FC3 128->5, 3 timesteps,
      all in [unit, sample] orientation -> no transposes.
"""

import numpy as np
import ml_dtypes

import concourse.bass as bass
import concourse.bacc as bacc
import concourse.tile as tile
import concourse.mybir as mybir

AF = mybir.ActivationFunctionType
ALU = mybir.AluOpType
BF16 = mybir.dt.bfloat16
F32 = mybir.dt.float32

NCORES = 8
B = 64  # samples per core


def _v(ap, p0, npart, dims, off=0):
    """View into an SBUF/PSUM tile AP: partition slice [p0, p0+npart) + custom free dims."""
    pitch = ap.ap[0][0]
    return bass.AP(
        tensor=ap.tensor,
        offset=ap.offset + p0 * pitch + off,
        ap=[[pitch, npart]] + [list(d) for d in dims],
    )


def _dv(ap, off, dims):
    """View into a DRAM tensor AP with custom dims."""
    return bass.AP(tensor=ap.tensor, offset=ap.offset + off, ap=[list(d) for d in dims])


def _emit(tc, io):
    nc = tc.nc
    from contextlib import ExitStack

    with ExitStack() as ctx:
        # ---------------- persistent buffers + weights ----------------
        wp = ctx.enter_context(tc.tile_pool(name="wts", bufs=1))
        w1sb = wp.tile([108, 128], BF16)
        nc.gpsimd.dma_start(w1sb[:, :], io["w1l"][:, :])
        w2sb = wp.tile([96, 192], BF16)
        nc.gpsimd.dma_start(w2sb[:, :], io["w2l"][:, :])
        w3asb = wp.tile([128, 384], BF16)
        nc.gpsimd.dma_start(w3asb[:, :], io["w3a"][:, :])
        w3bsb = wp.tile([64, 384], BF16)
        nc.gpsimd.dma_start(w3bsb[:, :], io["w3b"][:, :])
        wf2sb = wp.tile([128, 512], BF16)
        nc.gpsimd.dma_start(wf2sb[:, :], io["wf2"][:, :])
        wf3sb = wp.tile([128, 5], BF16)
        nc.gpsimd.dma_start(wf3sb[:, :], io["wf3"][:, :])
        b1sb = wp.tile([128, 1], F32)
        nc.gpsimd.dma_start(b1sb[:, :], io["b1"][:, :])
        b2sb = wp.tile([128, 1], F32)
        nc.gpsimd.dma_start(b2sb[:, :], io["b2"][:, :])
        b3sb = wp.tile([128, 1], F32)
        nc.gpsimd.dma_start(b3sb[:, :], io["b3"][:, :])

        mp = ctx.enter_context(tc.tile_pool(name="main", bufs=1))
        # conv1 pooled output, padded 26x26, partition 32q+c holds samples 16q..16q+15
        xpad2 = mp.tile([128, 16 * 676 + 4], BF16)
        for dims, off in [
            ([[676, 16], [1, 26]], 0),        # top row
            ([[676, 16], [1, 26]], 650),      # bottom row
            ([[676, 16], [26, 26]], 0),       # left col
            ([[676, 16], [26, 26]], 25),      # right col
            ([[1, 4]], 16 * 676),             # tail pad (im2col dx over-read)
        ]:
            nc.gpsimd.memset(_v(xpad2, 0, 128, dims, off), 0.0)
        # conv2 pooled output, padded 14x14, partition 64h+c holds samples 32h..32h+31
        xpad3 = mp.tile([128, 32 * 198 + 4], BF16)
        for dims, off in [
            ([[198, 32], [1, 14]], 0),        # top row
            ([[198, 32], [1, 14]], 182),      # bottom row
            ([[198, 32], [14, 14]], 0),       # left col
            ([[198, 32], [14, 14]], 13),      # right col
            ([[1, 4]], 32 * 198),             # tail pad (im2col dx over-read)
            ([[198, 32], [1, 2]], 196),       # per-sample slack (pitch 198 vs 196)
        ]:
            nc.gpsimd.memset(_v(xpad3, 0, 128, dims, off), 0.0)
        # conv3 pooled output (features): [128c, b*36 + hw]
        feat = mp.tile([128, B * 36], BF16)
        # conv2 im2col buffer: allocated up-front (fresh space, not recycled conv1 SBUF)
        # so its per-quarter DMAs can start as soon as xpad2 sample ranges are written.
        buf96 = mp.tile([96, B * 676 + 4], BF16)
        # LIF state lives in persistent space so its memsets run at t~0 instead
        # of waiting for conv-pool SBUF regions to free up.
        zeros256 = mp.tile([128, 256], F32)
        nc.gpsimd.memset(zeros256[:, :], 0.0)
        v1 = mp.tile([128, 256], F32)
        s1 = mp.tile([128, 256], BF16)
        nc.gpsimd.memset(v1[:, :], 0.0)
        v2 = mp.tile([128, 64], F32)
        nc.gpsimd.memset(v2[:, :], 0.0)
        s2 = mp.tile([128, 64], BF16)
        v3 = mp.tile([5, 64], F32)
        nc.gpsimd.memset(v3[:, :], 0.0)
        acc = mp.tile([5, 64], F32)
        nc.gpsimd.memset(acc[:, :], 0.0)

        # ---------------- conv1 ----------------
        # Block-diagonal stationary [108 = 4g x 27taps, 128 = 4g x 32ch] packs 4
        # sample-groups into one matmul (M=128); moving operand is the
        # host-precomputed im2col c1img [108, (16 slots x 48y x 48x)].
        # FC1 weights tile: chunks are streamed during the conv1/conv2 loop so
        # the 4.7MB load never monopolizes the DMA device right before conv3.
        fcw = ctx.enter_context(tc.tile_pool(name="fcw", bufs=1))
        wf1sb = fcw.tile([128, 18432], BF16)

        # conv1 and conv2 are interleaved per sample-quarter: conv2 block b only
        # needs xpad2 slots 4*(b%4)..4*(b%4)+3, so conv2 (PE-heavy) of quarter
        # qt runs while conv1 (DVE-heavy pooling) of quarter qt+1 proceeds.
        with (
            tc.tile_pool(name="c1imc", bufs=3) as c1i,
            tc.tile_pool(name="c1ps", bufs=4, space="PSUM") as c1p,
            tc.tile_pool(name="c1t", bufs=3) as c1t,
            tc.tile_pool(name="c2ps", bufs=4, space="PSUM") as c2p,
            tc.tile_pool(name="c2t", bufs=3) as c2t,
        ):
            imc_tiles = {}

            def _load_chunk(chunk):
                t = c1i.tile([108, 2 * 2304], BF16, tag="imc")
                for sub in range(2):
                    nc.sync.dma_start(
                        _v(t, 0, 108, [[1, 2304]], sub * 2304),
                        _dv(io["c1img"], (chunk * 2 + sub) * 2304, [[36864, 108], [1, 2304]]),
                    )
                imc_tiles[chunk] = t

            _load_chunk(0)
            _load_chunk(1)

            def _conv1_quarter(qt):
                for half in range(2):
                    chunk = qt * 2 + half
                    imc = imc_tiles.pop(chunk)
                    if chunk + 2 < 8:
                        _load_chunk(chunk + 2)  # prefetch ahead of buf96/wf1 traffic
                    for s in range(2):
                        stg = c1t.tile([128, 576], BF16, tag="stg")
                        slot = chunk * 2 + s
                        for yt in range(6):
                            ps = c1p.tile([128, 384], F32, tag="ps1")
                            rhs = _v(imc, 0, 108, [[1, 384]], s * 2304 + yt * 384)
                            nc.tensor.matmul(
                                ps[:, :], w1sb[0:108, 0:128], rhs, start=True, stop=True
                            )
                            # maxpool 2x2 on (8y, 48x) -> (4y, 24x) into staging
                            nc.vector.tensor_reduce(
                                _v(stg, 0, 128, [[24, 4], [1, 24]], yt * 96),
                                _v(ps, 0, 128, [[96, 4], [2, 24], [48, 2], [1, 2]]),
                                mybir.AxisListType.XY,
                                ALU.max,
                            )
                        dst = _v(xpad2, 0, 128, [[26, 24], [1, 24]], slot * 676 + 27)
                        nc.scalar.activation(dst, _v(stg, 0, 128, [[24, 24], [1, 24]]), AF.Relu, bias=b1sb[:, 0:1])
                    # buf96 for this slot-pair: pipelines with conv1 instead of
                    # batching at quarter end (keeps conv2's PE head unblocked)
                    for j in range(4):
                        bsrc = _v(xpad2, 32 * j, 32, [[1, 3], [1, 2 * 676]], chunk * 2 * 676)
                        bdst = _v(buf96, 0, 96, [[1, 2 * 676]], (j * 16 + chunk * 2) * 676)
                        nc.scalar.dma_start(bdst, bsrc)

            def _conv2_quarter(qt):
                for blk in (qt, qt + 4):
                    stg = c2t.tile([128, 576], BF16, tag="stg")
                    for bi in range(4):
                        b = blk * 4 + bi
                        for yh in range(2):
                            ps = c2p.tile([128, 288], F32, tag="ps2")
                            for h in range(2):
                                for dy in range(3):
                                    rhs = _v(
                                        buf96, 0, 96, [[26, 12], [1, 24]],
                                        (h * 32 + b) * 676 + yh * 312 + dy * 26,
                                    )
                                    nc.tensor.matmul(
                                        ps[64 * h : 64 * h + 64, :],
                                        w2sb[0:96, dy * 64 : dy * 64 + 64],
                                        rhs,
                                        start=(dy == 0),
                                        stop=(dy == 2),
                                        tile_position=(0, 64 * h),
                                    )
                            # pool (12y, 24x) -> (6y, 12x) into staging
                            nc.vector.tensor_reduce(
                                _v(stg, 0, 128, [[12, 6], [1, 12]], bi * 144 + yh * 72),
                                _v(ps, 0, 128, [[48, 6], [2, 12], [24, 2], [1, 2]]),
                                mybir.AxisListType.XY,
                                ALU.max,
                            )
                    dst = _v(xpad3, 0, 128, [[198, 4], [14, 12], [1, 12]], blk * 4 * 198 + 15)
                    nc.scalar.activation(dst, _v(stg, 0, 128, [[144, 4], [12, 12], [1, 12]]), AF.Relu, bias=b2sb[:, 0:1])
                # stream a quarter of the FC1 weights per qt iteration
                nc.gpsimd.dma_start(
                    _v(wf1sb, 0, 128, [[1, 4608]], qt * 4608),
                    _dv(io["wf1"], qt * 4608, [[18432, 128], [1, 4608]]),
                )

            # software-pipeline with 1-quarter skew: conv1(qt+1) is emitted
            # before conv2(qt) so the PE FIFO never stalls on buf96.
            _conv1_quarter(0)
            for qt in range(4):
                if qt + 1 < 4:
                    _conv1_quarter(qt + 1)
                _conv2_quarter(qt)

        # ---------------- conv3 (+ FC1 halves interleaved) ----------------
        cur1p = ctx.enter_context(tc.tile_pool(name="cur1p", bufs=1, space="PSUM"))
        cur1 = cur1p.tile([128, 256], F32)
        with (
            tc.tile_pool(name="c3buf", bufs=1) as c3b,
            tc.tile_pool(name="c3ps", bufs=4, space="PSUM") as c3p,
            tc.tile_pool(name="c3t", bufs=3) as c3t,
        ):
            for h in range(2):
                bufA = c3b.tile([128, 32 * 198 + 4], BF16, tag="A")
                bufB = c3b.tile([64, 32 * 198 + 4], BF16, tag="B")
                for ck in range(4):
                    off = ck * 8 * 198
                    nc.sync.dma_start(
                        _v(bufA, 0, 128, [[1, 8 * 198]], off),
                        _v(xpad3, 64 * h, 64, [[1, 2], [1, 8 * 198]], off),
                    )
                    nc.sync.dma_start(
                        _v(bufB, 0, 64, [[1, 8 * 198]], off),
                        _v(xpad3, 64 * h, 64, [[1, 8 * 198]], off + 2),
                    )
                for bq in range(4):
                    stg = c3t.tile([128, 288], BF16, tag="stg")
                    for bj in range(4):
                        bp = bq * 4 + bj
                        ps = c3p.tile([128, 288], F32, tag="ps3")
                        for dy in range(3):
                            dims = [[198, 2], [14, 12], [1, 12]]
                            off = bp * 2 * 198 + dy * 14
                            nc.tensor.matmul(
                                ps[:, :], w3asb[0:128, dy * 128 : dy * 128 + 128],
                                _v(bufA, 0, 128, dims, off),
                                start=(dy == 0), stop=False,
                            )
                            nc.tensor.matmul(
                                ps[:, :], w3bsb[0:64, dy * 128 : dy * 128 + 128],
                                _v(bufB, 0, 64, dims, off),
                                start=False, stop=(dy == 2),
                            )
                        # pool (2b, 12y, 12x) -> (2b, 6y, 6x): one XY-reduce per sample
                        for i in range(2):
                            nc.vector.tensor_reduce(
                                _v(stg, 0, 128, [[6, 6], [1, 6]], bj * 72 + i * 36),
                                _v(ps, 0, 128, [[24, 6], [2, 6], [12, 2], [1, 2]], i * 144),
                                mybir.AxisListType.XY,
                                ALU.max,
                            )
                    dst = _v(feat, 0, 128, [[1, 288]], (h * 32 + bq * 8) * 36)
                    nc.scalar.activation(dst, _v(stg, 0, 128, [[1, 288]]), AF.Relu, bias=b3sb[:, 0:1])
                # FC1 half for this h's 32 samples: unit-stationary, output in
                # [unit, sample] orientation (no transposes); overlaps conv3 h=1.
                for k in range(36):
                    for g in range(4):
                        nc.tensor.matmul(
                            cur1[:, 64 * g + 32 * h : 64 * g + 32 * h + 32],
                            wf1sb[:, k * 512 + 128 * g : k * 512 + 128 * (g + 1)],
                            _v(feat, 0, 128, [[36, 32]], k + h * 32 * 36),
                            start=(k == 0),
                            stop=(k == 35),
                        )

        # ---------------- LIF + FC2/FC3 ----------------
        with (
            tc.tile_pool(name="cur2p", bufs=2, space="PSUM") as cur2p,
            tc.tile_pool(name="liftmp", bufs=2) as dtp,
        ):

            def lif_step(v, cur, s_out):
                # v <- v + (cur - v)*0.5 ; s = (v >= 1) ; v <- 0 where s
                n = v.shape[1]
                d = dtp.tile([v.shape[0], n], F32, tag="d", name="d")
                nc.vector.tensor_tensor(d[:, :], cur[:, :], v[:, :], ALU.subtract)
                nc.vector.scalar_tensor_tensor(v[:, :], d[:, :], 0.5, v[:, :], ALU.mult, ALU.add)
                nc.vector.tensor_scalar(s_out[:, :], v[:, :], 1.0, None, ALU.is_ge)
                mask = s_out[:, :].bitcast(mybir.dt.uint16 if s_out.dtype == BF16 else mybir.dt.uint32)
                nc.vector.copy_predicated(v[:, :], mask, zeros256[0 : v.shape[0], 0 : n])

            for t in range(3):
                lif_step(v1, cur1, s1)
                cur2 = cur2p.tile([128, 64], F32, tag="cur2")
                for g in range(4):
                    nc.tensor.matmul(
                        cur2[:, :], wf2sb[:, g * 128 : g * 128 + 128], s1[:, 64 * g : 64 * g + 64],
                        start=(g == 0), stop=(g == 3),
                    )
                lif_step(v2, cur2, s2)
                cur3 = cur2p.tile([5, 64], F32, tag="cur3")
                nc.tensor.matmul(cur3[0:5, :], wf3sb[0:128, 0:5], s2[:, :], start=True, stop=True)
                s3 = dtp.tile([5, 64], F32, tag="s3")
                d3 = dtp.tile([5, 64], F32, tag="d3")
                nc.vector.tensor_tensor(d3[:, :], cur3[0:5, :], v3[:, :], ALU.subtract)
                nc.vector.scalar_tensor_tensor(v3[:, :], d3[:, :], 0.5, v3[:, :], ALU.mult, ALU.add)
                nc.vector.tensor_scalar(s3[:, :], v3[:, :], 1.0, None, ALU.is_ge)
                nc.vector.copy_predicated(v3[:, :], s3[:, :].bitcast(mybir.dt.uint32), zeros256[0:5, 0:64])
                nc.vector.tensor_tensor(acc[:, :], acc[:, :], s3[:, :], ALU.add)

            # acc/3 for acc in {0,1,2,3}: mult by fp32(1/3) matches true division except acc=3
            # (3*0.33333334 = 1.0000001) -> clamp with min(., 1.0) for exactness.
            nc.vector.tensor_scalar(acc[:, :], acc[:, :], float(np.float32(1.0) / np.float32(3.0)), 1.0, ALU.mult, ALU.min)
            nc.sync.dma_start(_dv(io["out"], 0, [[64, 5], [1, 64]]), acc[:, :])


def _build():
    nc = bacc.Bacc("TRN2", target_bir_lowering=False, debug=False, enable_asserts=True)
    io = {}

    def inp(name, shape, dt):
        io[name] = nc.dram_tensor(name, shape, dt, kind="ExternalInput").ap()

    inp("c1img", [108, 36864], BF16)
    inp("w1l", [108, 128], BF16)
    inp("w2l", [96, 192], BF16)
    inp("w3a", [128, 384], BF16)
    inp("w3b", [64, 384], BF16)
    inp("wf1", [128, 18432], BF16)
    inp("wf2", [128, 512], BF16)
    inp("wf3", [128, 5], BF16)
    inp("b1", [128, 1], F32)
    inp("b2", [128, 1], F32)
    inp("b3", [128, 1], F32)
    io["out"] = nc.dram_tensor("out", [5, 64], F32, kind="ExternalOutput").ap()

    import os
    unroll = int(os.environ.get("KERNEL_UNROLL", "1"))
    with tile.TileContext(nc) as tc:
        for _ in range(unroll):
            _emit(tc, io)
    nc.compile()
    return nc


def _fake_quant(w):
    w = np.asarray(w, np.float32)
    scale = np.float32(np.max(np.abs(w)) / np.float32(127.0))
    wq = np.clip(np.round(w / scale), -127.0, 127.0).astype(np.float32) * scale
    return wq.astype(np.float32)


def _bf16(a):
    return np.asarray(a, np.float32).astype(ml_dtypes.bfloat16)


def _prep_weights(conv1_w, conv1_b, conv2_w, conv2_b, conv3_w, conv3_b, W1, W2, W3):
    c1 = np.asarray(conv1_w, np.float32)  # [32, 3, 3, 3]
    c2 = np.asarray(conv2_w, np.float32)  # [64, 32, 3, 3]
    c3 = np.asarray(conv3_w, np.float32)  # [128, 64, 3, 3]

    # conv1 block-diagonal stationary: [108 = (g, c, dy, dx), 128 = (g, co)]
    w1l = np.zeros((108, 128), np.float32)
    wk = c1.transpose(1, 2, 3, 0).reshape(27, 32)  # [(c,dy,dx), co]
    for g in range(4):
        w1l[27 * g : 27 * g + 27, 32 * g : 32 * g + 32] = wk

    w2l = c2.transpose(1, 3, 2, 0).reshape(96, 192)  # [(c,dx), (dy,m)]
    w3x = c3.transpose(1, 3, 2, 0)  # [c, dx, dy, m]
    w3a = w3x[:, 0:2].reshape(128, 384)
    w3b = w3x[:, 2].reshape(64, 384)

    W1q = _fake_quant(W1)  # [512, 4608]
    W2q = _fake_quant(W2)  # [128, 512]
    W3q = _fake_quant(W3)  # [5, 128]

    # [c, k*512 + 128g + u] = W1q[128g + u, c*36 + k]  (unit-stationary FC1)
    wf1 = W1q.reshape(4, 128, 128, 36).transpose(2, 3, 0, 1).reshape(128, 36 * 512)
    wf2 = W2q.T.reshape(4, 128, 128).transpose(1, 0, 2).reshape(128, 512)
    wf3 = W3q.T.copy()  # [128, 5]

    return {
        "w1l": _bf16(w1l),
        "w2l": _bf16(w2l),
        "w3a": _bf16(w3a),
        "w3b": _bf16(w3b),
        "wf1": _bf16(wf1),
        "wf2": _bf16(wf2),
        "wf3": _bf16(wf3),
        "b1": np.tile(np.asarray(conv1_b, np.float32), 4).reshape(128, 1).copy(),
        "b2": np.tile(np.asarray(conv2_b, np.float32), 2).reshape(128, 1).copy(),
        "b3": np.asarray(conv3_b, np.float32).reshape(128, 1).copy(),
    }


_NC = None
LAST_RESULTS = None


def kernel(x, conv1_w, conv1_b, conv2_w, conv2_b, conv3_w, conv3_b, W1, W2, W3, _trace=False):
    global _NC, LAST_RESULTS
    if _NC is None:
        _NC = _build()

    wmap = _prep_weights(conv1_w, conv1_b, conv2_w, conv2_b, conv3_w, conv3_b, W1, W2, W3)

    x = np.asarray(x, np.float32)
    xp = np.zeros((512, 3, 50, 50), np.float32)
    xp[:, :, 1:49, 1:49] = x
    xpb = _bf16(xp)
    in_maps = []
    for i in range(NCORES):
        # host im2col: [108 = (g, c, dy, dx), 36864 = (slot s, y, x)]
        # row value at col (s,y,x) = xpad[64i + 16g + s, c, y+dy, x+dx]
        arr = xpb[B * i : B * (i + 1)].reshape(4, 16, 3, 50, 50)
        c1img = np.empty((108, 36864), ml_dtypes.bfloat16)
        for g in range(4):
            for c in range(3):
                for dy in range(3):
                    for dx in range(3):
                        r = 27 * g + 9 * c + 3 * dy + dx
                        c1img[r] = arr[g, :, c, dy : dy + 48, dx : dx + 48].reshape(-1)
        in_maps.append({"c1img": c1img, **wmap})

    from concourse.bass_utils import run_bass_kernel_spmd

    res = run_bass_kernel_spmd(_NC, in_maps, core_ids=list(range(NCORES)), trace=_trace)
    LAST_RESULTS = res
    out = np.concatenate([np.asarray(res.results[i]["out"]).T for i in range(NCORES)], axis=0)
    return np.ascontiguousarray(out.astype(np.float32))

